# revision 1
# baseline (speedup 1.0000x reference)
"""Multi-head causal attention (B=4, T=2048, C=1024, H=16) on 8 TRN2 NeuronCores.

Sharding: core c handles batch b=c//2 and head-group g=c%2 (8 heads = 4 pairs).
Per core: QKV projections for its 512 feature columns, causal attention for its
8 heads, partial out-projection. Host sums the two head-group partials per batch
and adds b_o.

All matmuls run in float32r (reduced-precision fp32, ~1.6e-4 rel err, 1 cyc/row
at moving dim >= 256 vs 4 cyc/row for fp32).
"""
import sys, types
import numpy as np
from contextlib import ExitStack

sys.path.insert(0, "/opt/trn_rl_repo")

import concourse.bass as bass
import concourse.tile as tile
from concourse import bacc, mybir
from concourse.bass_utils import run_bass_kernel_spmd

f32 = mybir.dt.float32
f32r = mybir.dt.float32r
EXP = mybir.ActivationFunctionType.Exp

C = 1024          # model dim
HG = 512          # per-core head-group feature width (8 heads x 64)
D = 64            # head dim
NPAIR = 4         # head pairs per core
NCC = C // 128    # contraction chunks (8)
SCALE = 0.125     # 1/sqrt(D)


def build_kernel(T):
    """Emit the per-core Bass program. T = sequence length (multiple of 512)."""
    NQT = T // 512    # q tiles of 512
    NKT = T // 128    # k tiles of 128

    nc = bacc.Bacc("TRN2", target_bir_lowering=False, debug=False, num_devices=8)

    xT = nc.dram_tensor("xT", [C, T], f32, kind="ExternalInput").ap()
    wq = nc.dram_tensor("wq", [C, HG], f32, kind="ExternalInput").ap()
    wk = nc.dram_tensor("wk", [C, HG], f32, kind="ExternalInput").ap()
    wv = nc.dram_tensor("wv", [C, HG], f32, kind="ExternalInput").ap()
    wo = nc.dram_tensor("wo", [HG, C], f32, kind="ExternalInput").ap()
    out = nc.dram_tensor("out", [NPAIR, T, C], f32, kind="ExternalOutput").ap()

    with tile.TileContext(nc) as tc, ExitStack() as ctx:
        # ---- SBUF pools (bytes/partition noted) ----
        p_xt = ctx.enter_context(tc.tile_pool(name="xt", bufs=NCC))          # 8x8K=64K
        p_w = ctx.enter_context(tc.tile_pool(name="w", bufs=2))              # qk pair weights 2x4K
        p_wv = ctx.enter_context(tc.tile_pool(name="wv", bufs=1))            # 16K
        p_wo = ctx.enter_context(tc.tile_pool(name="wo", bufs=1))            # 4K
        p_qk = ctx.enter_context(tc.tile_pool(name="qk", bufs=4))            # 4x8K=32K
        p_v = ctx.enter_context(tc.tile_pool(name="v", bufs=4))              # 4x8.5K
        p_phat = ctx.enter_context(tc.tile_pool(name="phat", bufs=2))        # 2x4K
        p_ctxT = ctx.enter_context(tc.tile_pool(name="ctxT", bufs=3))        # 3x2K
        p_cxs = ctx.enter_context(tc.tile_pool(name="cxs", bufs=3))          # 3x4K
        p_small = ctx.enter_context(tc.tile_pool(name="small", bufs=1))      # recip/bcast
        p_ostg = ctx.enter_context(tc.tile_pool(name="ostg", bufs=3))        # 3x2K
        p_ones = ctx.enter_context(tc.tile_pool(name="ones", bufs=1))
        # ---- PSUM pools: 4 + 2 + 2 = 8 banks ----
        ps_s = ctx.enter_context(tc.tile_pool(name="ps_s", bufs=2, space="PSUM"))    # [128,1024] x2
        ps_ctx = ctx.enter_context(tc.tile_pool(name="ps_ctx", bufs=1, space="PSUM"))
        ps_mm = ctx.enter_context(tc.tile_pool(name="ps_mm", bufs=2, space="PSUM"))

        # ---- constants + bulk loads ----
        ones_f = p_ones.tile([128, 1], f32)
        nc.vector.memset(ones_f, 1.0)

        # HAM warm-up: ~8us of dummy matmuls during the input DMAs so the
        # PE clock is at 2.4GHz when real work starts.
        warm = p_ostg.tile([128, 512], f32, tag="ostg")
        nc.vector.memset(warm, 0.0)
        warm_r = p_ctxT.tile([128, 512], f32r, tag="ctxT")
        nc.vector.tensor_copy(warm_r, warm)
        wps = ps_mm.tile([128, 512], f32, tag="mm")
        for i in range(24):
            nc.tensor.matmul(wps, warm_r[:, 0:128], warm_r,
                             start=(i == 0), stop=(i == 23))

        wv_sb = p_wv.tile([128, NCC, HG], f32r)
        nc.sync.dma_start(wv_sb, wv.rearrange("(cc p) f -> p cc f", p=128).bitcast(f32r))
        xt = []
        for cc in range(NCC):
            t_ = p_xt.tile([128, T], f32r, tag="xt")
            nc.sync.dma_start(t_, xT[cc * 128 : (cc + 1) * 128, :].bitcast(f32r))
            xt.append(t_)

        def load_wqk(p):
            """[128, 8, 128] tile: cc-chunks of W{q,k}[:, p*128:(p+1)*128]."""
            tq = p_w.tile([128, NCC, 128], f32r, tag="wq")
            tk = p_w.tile([128, NCC, 128], f32r, tag="wk")
            nc.scalar.dma_start(
                tq, wq[:, p * 128 : (p + 1) * 128].rearrange("(cc p) f -> p cc f", p=128).bitcast(f32r))
            nc.scalar.dma_start(
                tk, wk[:, p * 128 : (p + 1) * 128].rearrange("(cc p) f -> p cc f", p=128).bitcast(f32r))
            return tq, tk

        def load_wo(p):
            t_ = p_wo.tile([128, C], f32r, tag="wo")
            nc.scalar.dma_start(t_, wo[p * 128 : (p + 1) * 128, :].bitcast(f32r))
            return t_

        # ---- filler unit generators (PE work to hide under ACT-bound attention) ----
        v_groups = [None] * (NKT // 4)   # [128, 4, 8, 65] tiles, 4 k-tiles each

        def v_tile(j):
            g = v_groups[j // 4]
            assert g is not None, f"V group {j // 4} not emitted yet"
            return g[:, j % 4]

        v_sb = [None] * NKT

        def v_unit(j):
            def emit():
                ps = ps_mm.tile([128, HG], f32, tag="mm")
                for cc in range(NCC):
                    nc.tensor.matmul(
                        ps, xt[cc][:, j * 128 : (j + 1) * 128],
                        wv_sb[:, cc, :], start=(cc == 0), stop=(cc == NCC - 1))
                # V' layout [128, 8 heads, 65]: 64 V columns + a ones column so a
                # single M=65 ctx matmul also produces the softmax denominator.
                if j % 4 == 0:
                    v_groups[j // 4] = p_v.tile(
                        [128, 4, 8, D + 1], f32r, tag="v", name=f"vg{j // 4}")
                t_ = v_tile(j)
                nc.vector.tensor_copy(
                    t_[:, :, 0:D], ps.rearrange("p (h d) -> p h d", h=8))
                nc.vector.tensor_copy(
                    t_[:, :, D : D + 1], ones_f.to_broadcast([128, 8, 1]))
                v_sb[j] = t_
            return emit

        qkT = {}   # (('q'|'k'), pair) -> [128, T] tile

        def qk_unit(p, which, wtile, tt):
            def emit():
                key = (which, p)
                if key not in qkT:
                    qkT[key] = p_qk.tile([128, T], f32r, tag="qk", name=f"qk_{which}{p}")
                ps = ps_mm.tile([128, 512], f32, tag="mm")
                for cc in range(NCC):
                    nc.tensor.matmul(
                        ps, wtile[:, cc, :], xt[cc][:, tt * 512 : (tt + 1) * 512],
                        start=(cc == 0), stop=(cc == NCC - 1))
                nc.vector.tensor_copy(qkT[key][:, tt * 512 : (tt + 1) * 512], ps)
            return emit

        ctxT_store = {}  # (p, t) -> [128, 512] tile

        outdma_rr = [0]

        def outproj_unit(p, wo_tile, t, qq, half):
            def emit():
                ct = ctxT_store[(p, t)]
                stg = p_ostg.tile([128, 512], f32, tag="ostg")
                ps = ps_mm.tile([128, 512], f32, tag="mm")
                nc.tensor.matmul(
                    ps, ct[:, qq * 128 : (qq + 1) * 128],
                    wo_tile[:, half * 512 : (half + 1) * 512], start=True, stop=True)
                nc.vector.tensor_copy(stg, ps)
                if qq == 3 and half == 1:
                    ctxT_store.pop((p, t))
                outdma_rr[0] += 1
                nc.sync.dma_start(
                    out[p, t * 512 + qq * 128 : t * 512 + (qq + 1) * 128,
                        half * 512 : (half + 1) * 512], stg)
            return emit

        pending_norm = []
        wo_tiles = {}

        def make_norm(p, t, cxs):
            ct = p_ctxT.tile([128, 512], f32r, tag="ctxT", name=f"ct_{p}_{t}")
            ctxT_store[(p, t)] = ct
            def rep64(row):
                # [1,512] SBUF row -> [1, 64, 512] AP repeating the row 64x
                # (0-step on a free dim; partition dim must keep step!=0)
                return bass.AP(tensor=row.tensor, offset=row.offset,
                               ap=[list(row.ap[0]), [0, 64], list(row.ap[1])])
            state = {}
            def front():
                sc = p_small.tile([64, 16], f32, tag="scat")
                rc = p_small.tile([128, 1024], f32, tag="recip")
                bcab = p_small.tile([64, 1024], f32, tag="bcast")
                bc = bcab[:, 0:512]
                bc2 = bcab[:, 512:1024]
                # scatter denom row over 64 lanes for the reciprocal
                # (~7.5 cyc/elem serial per lane), gather, broadcast.
                nc.scalar.dma_start(sc, cxs[64:65, :])
                nc.vector.reciprocal(sc, sc)
                nc.scalar.dma_start(rc[64:65, :], sc)
                nc.scalar.dma_start(bc, rep64(rc[64:65, 0:512]))
                nc.scalar.dma_start(bc2, rep64(rc[64:65, 512:1024]))
                state["bc"], state["bc2"] = bc, bc2
            def back():
                tmpB = p_small.tile([64, 512], f32r, tag="tmpB")
                nc.vector.tensor_mul(ct[0:64, :], cxs[0:64, 0:512], state["bc"])
                nc.vector.tensor_mul(tmpB, cxs[0:64, 512:1024], state["bc2"])
                nc.scalar.dma_start(ct[64:128, :], tmpB)
            return p, t, front, back

        # ---- attention for one pair, pulling filler units between exp groups ----
        def attention(p, qt, kt, filler):
            for t in range(NQT):
                nk = 4 * (t + 1)
                norms = list(pending_norm)
                pending_norm.clear()
                for _, _, fr, _ in norms:
                    fr()
                cx = ps_ctx.tile([128, 1024], f32, tag="ctx")
                ctxA = cx[:, 0:512]
                ctxB = cx[:, 512:1024]
                for j in range(nk):
                    if j == 2:
                        for pp, tt, _, bk in norms:
                            bk()
                            if pp in wo_tiles:
                                for qq in range(4):
                                    for half in range(2):
                                        filler.append(outproj_unit(
                                            pp, wo_tiles[pp], tt, qq, half))
                    # S^T for both heads, row-tiled (contraction d=64 each)
                    sps = ps_s.tile([128, 1024], f32, tag="s")
                    nc.tensor.matmul(
                        sps[:, 0:512], kt[0:64, j * 128 : (j + 1) * 128],
                        qt[0:64, t * 512 : (t + 1) * 512],
                        start=True, stop=True, tile_position=(0, 0))
                    nc.tensor.matmul(
                        sps[:, 512:1024], kt[64:128, j * 128 : (j + 1) * 128],
                        qt[64:128, t * 512 : (t + 1) * 512],
                        start=True, stop=True, tile_position=(64, 0))
                    # exp(scale * S^T) for both heads in one ACT instruction
                    ph = p_phat.tile([128, 1024], f32r, tag="phat")
                    nc.scalar.activation(ph, sps, EXP, scale=SCALE)
                    # causal zeroing on diagonal-crossing blocks (k0 > q0 part)
                    off = j * 128 - t * 512
                    if off + 127 > 0:  # block crosses the diagonal
                        for h in range(2):
                            nc.gpsimd.affine_select(
                                out=ph[:, h * 512 : (h + 1) * 512],
                                in_=ph[:, h * 512 : (h + 1) * 512],
                                compare_op=mybir.AluOpType.is_ge,
                                fill=0.0, base=-off,
                                pattern=[[1, 512]], channel_multiplier=-1)
                    # ctx'^T accumulation: one M=65 matmul per head gives
                    # rows 0:64 = ctx^T and row 64 = softmax denominator
                    # (V' ones column). Single accumulation group per bank.
                    st, sp = (j == 0), (j == nk - 1)
                    assert v_sb[j] is not None, f"V tile {j} not emitted yet"
                    nc.tensor.matmul(ctxA[0:65, :], v_sb[j][:, 2 * p, :],
                                     ph[:, 0:512], start=st, stop=sp)
                    nc.tensor.matmul(ctxB[0:65, :], v_sb[j][:, 2 * p + 1, :],
                                     ph[:, 512:1024], start=st, stop=sp)
                    if filler and j >= (3 if norms else 1):
                        filler.pop(0)()
                # Evict unnormalized ctx' (rows 0:64 ctx, row 64 denom) to
                # SBUF in ONE DVE copy so the psum bank frees immediately.
                # The multi-hop normalize is deferred into the NEXT q-tile
                # iteration (front half at its start, muls at its middle) so
                # its DMA latency never heads the DVE queue.
                cxs = p_cxs.tile([128, 1024], f32, tag="cxs")
                nc.vector.tensor_copy(cxs[0:65, :], cx[0:65, :])
                pending_norm.append(make_norm(p, t, cxs))
                if filler:
                    filler.pop(0)()

        # ================= emission schedule =================
        # V tiles 0..3 + pair-0 Q/K upfront; rest of V and later pairs' proj
        # and out-proj are filler inside the ACT-bound attention loops.
        for j in range(4 * 1):
            v_unit(j)()
        w0q, w0k = load_wqk(0)
        for tt in range(NQT):
            qk_unit(0, "q", w0q, tt)()
            qk_unit(0, "k", w0k, tt)()

        for p in range(NPAIR):
            filler = []
            if p == 0:
                for j in range(4, NKT):
                    filler.append(v_unit(j))
            if p + 1 < NPAIR:
                wq_t, wk_t = load_wqk(p + 1)
                for tt in range(NQT):
                    filler.append(qk_unit(p + 1, "q", wq_t, tt))
                    filler.append(qk_unit(p + 1, "k", wk_t, tt))
            wo_tiles[p] = load_wo(p)
            attention(p, qkT[("q", p)], qkT[("k", p)], filler)
            for u in filler:  # drain any leftovers
                u()
            qkT.pop(("q", p)), qkT.pop(("k", p))
        # tail: last tile's normalize + its out-projection
        for pp, tt, fr, bk in pending_norm:
            fr(); bk()
            for qq in range(4):
                for half in range(2):
                    outproj_unit(pp, wo_tiles[pp], tt, qq, half)()
        pending_norm.clear()

    nc.compile()
    return nc


_NC_CACHE = {}


def _get_nc(T):
    if T not in _NC_CACHE:
        _NC_CACHE[T] = build_kernel(T)
    return _NC_CACHE[T]


def make_in_maps(x, W_q, W_k, W_v, W_o):
    B, T, _ = x.shape
    in_maps = []
    for c in range(8):
        b, g = c // 2, c % 2
        cols = slice(g * HG, (g + 1) * HG)
        in_maps.append({
            "xT": np.ascontiguousarray(x[b].T),
            "wq": np.ascontiguousarray(W_q[:, cols]),
            "wk": np.ascontiguousarray(W_k[:, cols]),
            "wv": np.ascontiguousarray(W_v[:, cols]),
            "wo": np.ascontiguousarray(W_o[cols, :]),
        })
    return in_maps


def kernel(x, W_q, W_k, W_v, W_o, b_o):
    x = np.asarray(x, dtype=np.float32)
    B, T, C_ = x.shape
    nc = _get_nc(T)
    in_maps = make_in_maps(x, np.asarray(W_q), np.asarray(W_k), np.asarray(W_v),
                           np.asarray(W_o))
    res = run_bass_kernel_spmd(nc, in_maps, core_ids=list(range(8)))
    out = np.empty((B, T, C_), dtype=np.float32)
    for b in range(B):
        pa = res.results[2 * b]["out"].sum(axis=0)
        pb = res.results[2 * b + 1]["out"].sum(axis=0)
        out[b] = pa + pb + np.asarray(b_o, dtype=np.float32)[None, :]
    return out



# revision 5
# speedup vs baseline: 1.4167x; 1.4167x over previous
"""Multi-head causal attention (B=4, T=2048, C=1024, H=16) on 8 TRN2 NeuronCores.

Sharding: core c handles batch b=c//2 and head-group g=c%2 (8 heads = 4 pairs).
Per core: QKV projections for its 512 feature columns, causal attention for its
8 heads, out-projection accumulated over the 4 head pairs in PSUM. Host sums
the two head-group partials per batch and adds b_o.

All operands are bf16 (1 cyc/row matmuls + FWL weight loads + half the DMA
bytes); accumulation stays fp32 in PSUM.
"""
import sys
import numpy as np
from contextlib import ExitStack

sys.path.insert(0, "/opt/trn_rl_repo")

import concourse.bass as bass
import concourse.tile as tile
from concourse import bacc, mybir
from concourse.bass_utils import run_bass_kernel_spmd

f32 = mybir.dt.float32
bf16 = mybir.dt.bfloat16
EXP = mybir.ActivationFunctionType.Exp

C = 1024          # model dim
HG = 512          # per-core head-group feature width (8 heads x 64)
D = 64            # head dim
DV = 66           # V' row: 64 V cols + ones col + pad (even => 4B-aligned heads)
NPAIR = 4         # head pairs per core
NCC = C // 128    # contraction chunks (8)
SCALE = 0.125     # 1/sqrt(D)


def build_kernel(T):
    """Emit the per-core Bass program. T = sequence length (multiple of 512)."""
    NQT = T // 512    # q tiles of 512
    NKT = T // 128    # k tiles of 128

    nc = bacc.Bacc("TRN2", target_bir_lowering=False, debug=False, num_devices=8)

    xT = nc.dram_tensor("xT", [C, T], bf16, kind="ExternalInput").ap()
    wq = nc.dram_tensor("wq", [C, HG], bf16, kind="ExternalInput").ap()
    wk = nc.dram_tensor("wk", [C, HG], bf16, kind="ExternalInput").ap()
    wv = nc.dram_tensor("wv", [C, HG], bf16, kind="ExternalInput").ap()
    wo = nc.dram_tensor("wo", [HG, C], bf16, kind="ExternalInput").ap()
    out = nc.dram_tensor("out", [T, C], bf16, kind="ExternalOutput").ap()

    with tile.TileContext(nc) as tc, ExitStack() as ctx:
        # ---- SBUF pools (bytes/partition noted) ----
        p_xt = ctx.enter_context(tc.tile_pool(name="xt", bufs=NCC))          # 8x4K=32K
        p_w = ctx.enter_context(tc.tile_pool(name="w", bufs=2))              # qk pair weights 2x2x2K
        p_wv = ctx.enter_context(tc.tile_pool(name="wv", bufs=1))            # 8K
        p_wo = ctx.enter_context(tc.tile_pool(name="wo", bufs=NPAIR))        # 4x2K
        p_qk = ctx.enter_context(tc.tile_pool(name="qk", bufs=4))            # 4x4K
        p_v = ctx.enter_context(tc.tile_pool(name="v", bufs=4))              # 4x4.2K
        p_phat = ctx.enter_context(tc.tile_pool(name="phat", bufs=3))        # 3x2K
        p_ctxT = ctx.enter_context(tc.tile_pool(name="ctxT", bufs=4 * NPAIR))  # 16x1K
        p_cxs = ctx.enter_context(tc.tile_pool(name="cxs", bufs=3))          # 3x2K
        p_small = ctx.enter_context(tc.tile_pool(name="small", bufs=1))      # recip/bcast
        p_ostg = ctx.enter_context(tc.tile_pool(name="ostg", bufs=3))        # 3x2K
        p_ones = ctx.enter_context(tc.tile_pool(name="ones", bufs=1))
        # ---- PSUM pools: 4 + 2 + 2 = 8 banks ----
        ps_s = ctx.enter_context(tc.tile_pool(name="ps_s", bufs=2, space="PSUM"))    # [128,1024] x2
        ps_ctx = ctx.enter_context(tc.tile_pool(name="ps_ctx", bufs=1, space="PSUM"))
        ps_mm = ctx.enter_context(tc.tile_pool(name="ps_mm", bufs=2, space="PSUM"))

        # ---- constants + bulk loads ----
        ones_f = p_ones.tile([128, 1], bf16)
        nc.vector.memset(ones_f, 1.0)

        # HAM warm-up: ~8us of dummy matmuls during the input DMAs so the
        # PE clock is at 2.4GHz when real work starts.
        warm = p_ostg.tile([128, 512], bf16, tag="ostg")
        nc.vector.memset(warm, 0.0)
        wps = ps_mm.tile([128, 512], f32, tag="mm")
        for i in range(40):
            nc.tensor.matmul(wps, warm[:, 0:128], warm,
                             start=(i == 0), stop=(i == 39))

        wv_sb = p_wv.tile([128, NCC, HG], bf16)
        nc.sync.dma_start(wv_sb, wv.rearrange("(cc p) f -> p cc f", p=128))
        xt = []
        for cc in range(NCC):
            t_ = p_xt.tile([128, T], bf16, tag="xt")
            nc.sync.dma_start(t_, xT[cc * 128 : (cc + 1) * 128, :])
            xt.append(t_)

        def load_wqk(p):
            """[128, 8, 128] tile: cc-chunks of W{q,k}[:, p*128:(p+1)*128]."""
            tq = p_w.tile([128, NCC, 128], bf16, tag="wq")
            tk = p_w.tile([128, NCC, 128], bf16, tag="wk")
            nc.scalar.dma_start(
                tq, wq[:, p * 128 : (p + 1) * 128].rearrange("(cc p) f -> p cc f", p=128))
            nc.scalar.dma_start(
                tk, wk[:, p * 128 : (p + 1) * 128].rearrange("(cc p) f -> p cc f", p=128))
            return tq, tk

        def load_wo(p):
            t_ = p_wo.tile([128, C], bf16, tag="wo", name=f"wo{p}")
            nc.scalar.dma_start(t_, wo[p * 128 : (p + 1) * 128, :])
            return t_

        # ---- filler unit generators (PE work to hide under ACT-bound attention) ----
        v_groups = [None] * (NKT // 4)   # [128, 4, 8, DV] tiles, 4 k-tiles each

        def v_tile(j):
            g = v_groups[j // 4]
            assert g is not None, f"V group {j // 4} not emitted yet"
            return g[:, j % 4]

        v_sb = [None] * NKT

        def v_unit(j):
            def emit():
                ps = ps_mm.tile([128, HG], f32, tag="mm")
                for cc in range(NCC):
                    nc.tensor.matmul(
                        ps, xt[cc][:, j * 128 : (j + 1) * 128],
                        wv_sb[:, cc, :], start=(cc == 0), stop=(cc == NCC - 1))
                # V' layout [128, 8 heads, DV]: 64 V columns + a ones column so a
                # single M=65 ctx matmul also produces the softmax denominator.
                if j % 4 == 0:
                    v_groups[j // 4] = p_v.tile(
                        [128, 4, 8, DV], bf16, tag="v", name=f"vg{j // 4}")
                t_ = v_tile(j)
                nc.vector.tensor_copy(
                    t_[:, :, 0:D], ps.rearrange("p (h d) -> p h d", h=8))
                nc.vector.tensor_copy(
                    t_[:, :, D : DV], ones_f.to_broadcast([128, 8, DV - D]))
                v_sb[j] = t_
            return emit

        qkT = {}   # (('q'|'k'), pair) -> [128, T] tile

        def qk_unit(p, which, wtile, tt):
            def emit():
                key = (which, p)
                if key not in qkT:
                    qkT[key] = p_qk.tile([128, T], bf16, tag="qk", name=f"qk_{which}{p}")
                ps = ps_mm.tile([128, 512], f32, tag="mm")
                for cc in range(NCC):
                    nc.tensor.matmul(
                        ps, wtile[:, cc, :], xt[cc][:, tt * 512 : (tt + 1) * 512],
                        start=(cc == 0), stop=(cc == NCC - 1))
                nc.vector.tensor_copy(qkT[key][:, tt * 512 : (tt + 1) * 512], ps)
            return emit

        ctxT_store = {}  # (p, t) -> [128, 512] bf16 tile (normalized ctx^T)
        wo_tiles = {}

        def outproj_unit(t, qq):
            """out[t*512+qq*128 : +128, :] = sum_p ctxT[p,t][:,qq]^T @ wo_p."""
            def emit():
                stg = p_ostg.tile([128, 1024], bf16, tag="ostg")
                for half in range(2):
                    ps = ps_mm.tile([128, 512], f32, tag="mm")
                    for p in range(NPAIR):
                        nc.tensor.matmul(
                            ps, ctxT_store[(p, t)][:, qq * 128 : (qq + 1) * 128],
                            wo_tiles[p][:, half * 512 : (half + 1) * 512],
                            start=(p == 0), stop=(p == NPAIR - 1))
                    nc.vector.tensor_copy(stg[:, half * 512 : (half + 1) * 512], ps)
                nc.sync.dma_start(
                    out[t * 512 + qq * 128 : t * 512 + (qq + 1) * 128, :], stg)
            return emit

        pending_norm = []

        def make_norm(p, t, cxs):
            ct = p_ctxT.tile([128, 512], bf16, tag="ctxT", name=f"ct_{p}_{t}")
            ctxT_store[(p, t)] = ct
            def rep64(row):
                # [1,512] SBUF row -> [1, 64, 512] AP repeating the row 64x
                # (0-step on a free dim; partition dim must keep step!=0)
                return bass.AP(tensor=row.tensor, offset=row.offset,
                               ap=[list(row.ap[0]), [0, 64], list(row.ap[1])])
            state = {}
            def front():
                sc = p_small.tile([64, 16], bf16, tag="scat")
                rc = p_small.tile([128, 1024], bf16, tag="recip")
                bcab = p_small.tile([64, 1024], bf16, tag="bcast")
                bc = bcab[:, 0:512]
                bc2 = bcab[:, 512:1024]
                # scatter denom row over 64 lanes for the reciprocal
                # (serial per lane), gather, broadcast.
                nc.scalar.dma_start(sc, cxs[64:65, :])
                with nc.allow_low_precision(reason="bf16 softmax recip, tol 2e-2"):
                    nc.vector.reciprocal(sc, sc)
                nc.scalar.dma_start(rc[64:65, :], sc)
                nc.scalar.dma_start(bc, rep64(rc[64:65, 0:512]))
                nc.scalar.dma_start(bc2, rep64(rc[64:65, 512:1024]))
                state["bc"], state["bc2"] = bc, bc2
            def back():
                tmpB = p_small.tile([64, 512], bf16, tag="tmpB")
                nc.vector.tensor_mul(ct[0:64, :], cxs[0:64, 0:512], state["bc"])
                nc.vector.tensor_mul(tmpB, cxs[0:64, 512:1024], state["bc2"])
                nc.scalar.dma_start(ct[64:128, :], tmpB)
            return p, t, front, back

        # ---- attention for one pair, pulling filler units between exp groups ----
        def attention(p, qt, kt, filler):
            for t in range(NQT):
                nk = 4 * (t + 1)
                norms = list(pending_norm)
                pending_norm.clear()
                for _, _, fr, _ in norms:
                    fr()
                cx = ps_ctx.tile([128, 1024], f32, tag="ctx")
                ctxA = cx[:, 0:512]
                ctxB = cx[:, 512:1024]
                for j in range(nk):
                    if j == 2:
                        for pp, tt, _, bk in norms:
                            bk()
                            if pp == NPAIR - 1:
                                for qq in range(4):
                                    filler.append(outproj_unit(tt, qq))
                    # causal narrowing: columns q < off are fully masked for
                    # this k-tile -> skip them in S, exp and ctx.
                    off = max(0, j * 128 - t * 512)
                    W = 512 - off
                    qs = t * 512 + off
                    # S^T for both heads, row-tiled (contraction d=64 each)
                    sps = ps_s.tile([128, 1024], f32, tag="s")
                    nc.tensor.matmul(
                        sps[:, off : 512], kt[0:64, j * 128 : (j + 1) * 128],
                        qt[0:64, qs : (t + 1) * 512],
                        start=True, stop=True, tile_position=(0, 0))
                    nc.tensor.matmul(
                        sps[:, 512 + off : 1024], kt[64:128, j * 128 : (j + 1) * 128],
                        qt[64:128, qs : (t + 1) * 512],
                        start=True, stop=True, tile_position=(64, 0))
                    # exp(scale * S^T) for both heads in one ACT instruction
                    # ([128, 2, W] AP skips the masked prefix columns)
                    ph = p_phat.tile([128, 1024], bf16, tag="phat")
                    nc.scalar.activation(
                        ph.rearrange("p (h w) -> p h w", h=2)[:, :, off:512],
                        sps.rearrange("p (h w) -> p h w", h=2)[:, :, off:512],
                        EXP, scale=SCALE)
                    # causal zeroing on the 128-col diagonal slab (q in
                    # [off, off+128)): standard lower-triangular mask.
                    if j * 128 + 127 > t * 512:  # block crosses the diagonal
                        oe = min(off + 128, 512)
                        for h in range(2):
                            nc.gpsimd.affine_select(
                                out=ph[:, h * 512 + off : h * 512 + oe],
                                in_=ph[:, h * 512 + off : h * 512 + oe],
                                compare_op=mybir.AluOpType.is_ge,
                                fill=0.0, base=0,
                                pattern=[[1, oe - off]], channel_multiplier=-1)
                    # ctx'^T accumulation: one M=DV matmul per head gives
                    # rows 0:64 = ctx^T and row 64 = softmax denominator
                    # (V' ones column). Single accumulation group per bank.
                    st, sp = (j == 0), (j == nk - 1)
                    assert v_sb[j] is not None, f"V tile {j} not emitted yet"
                    nc.tensor.matmul(ctxA[0:DV, off:512], v_sb[j][:, 2 * p, :],
                                     ph[:, off : 512], start=st, stop=sp)
                    nc.tensor.matmul(ctxB[0:DV, off:512], v_sb[j][:, 2 * p + 1, :],
                                     ph[:, 512 + off : 1024], start=st, stop=sp)
                    if filler and j >= (3 if norms else 1):
                        filler.pop(0)()
                # Evict unnormalized ctx' (rows 0:64 ctx, row 64 denom) to
                # SBUF in ONE DVE copy so the psum bank frees immediately.
                # The multi-hop normalize is deferred into the NEXT q-tile
                # iteration (front half at its start, muls at its middle) so
                # its DMA latency never heads the DVE queue.
                cxs = p_cxs.tile([128, 1024], bf16, tag="cxs")
                nc.vector.tensor_copy(cxs[0:65, :], cx[0:65, :])
                pending_norm.append(make_norm(p, t, cxs))
                if filler:
                    filler.pop(0)()

        # ================= emission schedule =================
        # V tiles 0..3 + pair-0 Q/K upfront; rest of V and later pairs' proj
        # and the accumulated out-proj are filler inside the attention loops.
        for j in range(4 * 1):
            v_unit(j)()
        w0q, w0k = load_wqk(0)
        for tt in range(NQT):
            qk_unit(0, "q", w0q, tt)()
            qk_unit(0, "k", w0k, tt)()

        for p in range(NPAIR):
            filler = []
            if p == 0:
                for j in range(4, NKT):
                    filler.append(v_unit(j))
            if p + 1 < NPAIR:
                wq_t, wk_t = load_wqk(p + 1)
                for tt in range(NQT):
                    filler.append(qk_unit(p + 1, "q", wq_t, tt))
                    filler.append(qk_unit(p + 1, "k", wk_t, tt))
            wo_tiles[p] = load_wo(p)
            attention(p, qkT[("q", p)], qkT[("k", p)], filler)
            for u in filler:  # drain any leftovers
                u()
            qkT.pop(("q", p)), qkT.pop(("k", p))
        # tail: last tile's normalize + its out-projection
        for pp, tt, fr, bk in pending_norm:
            fr(); bk()
            if pp == NPAIR - 1:
                for qq in range(4):
                    outproj_unit(tt, qq)()
        pending_norm.clear()

    nc.compile()
    return nc


_NC_CACHE = {}


def _get_nc(T):
    if T not in _NC_CACHE:
        _NC_CACHE[T] = build_kernel(T)
    return _NC_CACHE[T]


def _bf16(a):
    import ml_dtypes
    return np.ascontiguousarray(a).astype(ml_dtypes.bfloat16)


def make_in_maps(x, W_q, W_k, W_v, W_o):
    B, T, _ = x.shape
    in_maps = []
    for c in range(8):
        b, g = c // 2, c % 2
        cols = slice(g * HG, (g + 1) * HG)
        in_maps.append({
            "xT": _bf16(np.asarray(x[b]).T),
            "wq": _bf16(np.asarray(W_q)[:, cols]),
            "wk": _bf16(np.asarray(W_k)[:, cols]),
            "wv": _bf16(np.asarray(W_v)[:, cols]),
            "wo": _bf16(np.asarray(W_o)[cols, :]),
        })
    return in_maps


def kernel(x, W_q, W_k, W_v, W_o, b_o):
    x = np.asarray(x, dtype=np.float32)
    B, T, C_ = x.shape
    nc = _get_nc(T)
    in_maps = make_in_maps(x, W_q, W_k, W_v, W_o)
    res = run_bass_kernel_spmd(nc, in_maps, core_ids=list(range(8)))
    out = np.empty((B, T, C_), dtype=np.float32)
    bo = np.asarray(b_o, dtype=np.float32)[None, :]
    for b in range(B):
        pa = np.asarray(res.results[2 * b]["out"]).astype(np.float32)
        pb = np.asarray(res.results[2 * b + 1]["out"]).astype(np.float32)
        out[b] = pa + pb + bo
    return out


# revision 13
# speedup vs baseline: 1.4781x; 1.0433x over previous
"""Multi-head causal attention (B=4, T=2048, C=1024, H=16) on 8 TRN2 NeuronCores.

Sharding: core c handles batch b=c//2 and head-group g=c%2 (8 heads = 4 pairs).
Per core: QKV projections for its 512 feature columns, causal attention for its
8 heads, out-projection accumulated over the 4 head pairs in PSUM. Host sums
the two head-group partials per batch and adds b_o.

All operands are bf16 (1 cyc/row matmuls + FWL weight loads + half the DMA
bytes); accumulation stays fp32 in PSUM.
"""
import sys
import numpy as np
from contextlib import ExitStack

sys.path.insert(0, "/opt/trn_rl_repo")

import concourse.bass as bass
import concourse.tile as tile
from concourse import bacc, mybir
from concourse.bass_utils import run_bass_kernel_spmd

f32 = mybir.dt.float32
bf16 = mybir.dt.bfloat16
fp8 = mybir.dt.float8e4
DR = mybir.MatmulPerfMode.DoubleRow
EXP = mybir.ActivationFunctionType.Exp

C = 1024          # model dim
HG = 512          # per-core head-group feature width (8 heads x 64)
D = 64            # head dim
DV = 66           # V' row: 64 V cols + ones col + pad (even => 4B-aligned heads)
NPAIR = 4         # head pairs per core
NCC = C // 128    # contraction chunks (8)
WS = 32.0         # host-side W_q/W_k fp8 scale (absorbed into the exp scale)
SCALE = 0.125     # 1/sqrt(D)
SCALE_S = SCALE / (WS * WS)   # exp scale for fp8-scaled Q/K scores


def build_kernel(T):
    """Emit the per-core Bass program. T = sequence length (multiple of 512)."""
    NQT = T // 512    # q tiles of 512
    NKT = T // 128    # k tiles of 128

    nc = bacc.Bacc("TRN2", target_bir_lowering=False, debug=False, num_devices=8)

    xT = nc.dram_tensor("xT", [C, T], bf16, kind="ExternalInput").ap()
    xT8 = nc.dram_tensor("xT8", [C, T], fp8, kind="ExternalInput").ap()
    wq = nc.dram_tensor("wq", [C, HG], fp8, kind="ExternalInput").ap()
    wk = nc.dram_tensor("wk", [C, HG], fp8, kind="ExternalInput").ap()
    wv = nc.dram_tensor("wv", [C, HG], bf16, kind="ExternalInput").ap()
    wo = nc.dram_tensor("wo", [HG, C], bf16, kind="ExternalInput").ap()
    out = nc.dram_tensor("out", [T, C], bf16, kind="ExternalOutput").ap()

    with tile.TileContext(nc) as tc, ExitStack() as ctx:
        # ---- SBUF pools (bytes/partition noted) ----
        p_xt = ctx.enter_context(tc.tile_pool(name="xt", bufs=NCC))          # 8x4K=32K
        p_w = ctx.enter_context(tc.tile_pool(name="w", bufs=2))              # qk pair weights 2x2x2K
        p_wv = ctx.enter_context(tc.tile_pool(name="wv", bufs=1))            # 8K
        p_wo = ctx.enter_context(tc.tile_pool(name="wo", bufs=NPAIR))        # 4x2K
        p_qk = ctx.enter_context(tc.tile_pool(name="qk", bufs=4))            # 4x4K
        p_v = ctx.enter_context(tc.tile_pool(name="v", bufs=4))              # 4x4.2K
        p_phat = ctx.enter_context(tc.tile_pool(name="phat", bufs=3))        # 3x2K
        p_ctxT = ctx.enter_context(tc.tile_pool(name="ctxT", bufs=4 * NPAIR))  # 16x1K
        p_cxs = ctx.enter_context(tc.tile_pool(name="cxs", bufs=3))          # 3x2K
        p_small = ctx.enter_context(tc.tile_pool(name="small", bufs=1))      # recip/bcast
        p_ostg = ctx.enter_context(tc.tile_pool(name="ostg", bufs=3))        # 3x2K
        p_ones = ctx.enter_context(tc.tile_pool(name="ones", bufs=1))
        # ---- PSUM pools: 4 + 2 + 2 = 8 banks ----
        ps_s = ctx.enter_context(tc.tile_pool(name="ps_s", bufs=2, space="PSUM"))    # [128,1024] x2
        ps_ctx = ctx.enter_context(tc.tile_pool(name="ps_ctx", bufs=1, space="PSUM"))
        ps_mm = ctx.enter_context(tc.tile_pool(name="ps_mm", bufs=2, space="PSUM"))

        # ---- constants + bulk loads ----
        ones_f = p_ones.tile([128, 1], bf16)
        nc.vector.memset(ones_f, 1.0)

        # HAM warm-up: ~5us of dummy matmuls during the input DMAs so the
        # PE clock is at 2.4GHz when real work starts.
        warm = p_ostg.tile([128, 512], bf16, tag="ostg")
        nc.vector.memset(warm, 0.0)
        wps = ps_mm.tile([128, 512], f32, tag="mm")
        for i in range(24):
            nc.tensor.matmul(wps, warm[:, 0:128], warm,
                             start=(i == 0), stop=(i == 23))

        wv_sb = p_wv.tile([128, NCC, HG], bf16)
        nc.sync.dma_start(wv_sb, wv.rearrange("(cc p) f -> p cc f", p=128))
        xt8 = p_xt.tile([128, NCC, T], fp8, tag="xt8", bufs=1)
        nc.sync.dma_start(xt8, xT8.rearrange("(cc p) t -> p cc t", p=128))
        xt = []
        for cc in range(NCC):
            t_ = p_xt.tile([128, T], bf16, tag="xt")
            nc.sync.dma_start(t_, xT[cc * 128 : (cc + 1) * 128, :])
            xt.append(t_)

        def load_wqk(p):
            """[128, 8, 128] fp8 tile: cc-chunks of W{q,k}[:, p*128:(p+1)*128]."""
            tq = p_w.tile([128, NCC, 128], fp8, tag="wq")
            tk = p_w.tile([128, NCC, 128], fp8, tag="wk")
            nc.scalar.dma_start(
                tq, wq[:, p * 128 : (p + 1) * 128].rearrange("(cc p) f -> p cc f", p=128))
            nc.scalar.dma_start(
                tk, wk[:, p * 128 : (p + 1) * 128].rearrange("(cc p) f -> p cc f", p=128))
            return tq, tk

        def load_wo(p):
            t_ = p_wo.tile([128, C], bf16, tag="wo", name=f"wo{p}")
            nc.scalar.dma_start(t_, wo[p * 128 : (p + 1) * 128, :])
            return t_

        # ---- filler unit generators (PE work to hide under ACT-bound attention) ----
        v_groups = [None] * (NKT // 4)   # [128, 4, 8, DV] tiles, 4 k-tiles each

        def v_tile(j):
            g = v_groups[j // 4]
            assert g is not None, f"V group {j // 4} not emitted yet"
            return g[:, j % 4]

        v_sb = [None] * NKT

        def v_unit(j):
            def emit():
                ps = ps_mm.tile([128, HG], f32, tag="mm")
                for cc in range(NCC):
                    nc.tensor.matmul(
                        ps, xt[cc][:, j * 128 : (j + 1) * 128],
                        wv_sb[:, cc, :], start=(cc == 0), stop=(cc == NCC - 1))
                # V' layout [128, 8 heads, DV]: 64 V columns + a ones column so a
                # single M=65 ctx matmul also produces the softmax denominator.
                if j % 4 == 0:
                    v_groups[j // 4] = p_v.tile(
                        [128, 4, 8, DV], bf16, tag="v", name=f"vg{j // 4}")
                t_ = v_tile(j)
                nc.vector.tensor_copy(
                    t_[:, :, 0:D], ps.rearrange("p (h d) -> p h d", h=8))
                nc.vector.tensor_copy(
                    t_[:, :, D : DV], ones_f.to_broadcast([128, 8, DV - D]))
                v_sb[j] = t_
            return emit

        qkT = {}   # (('q'|'k'), pair) -> [128, T] tile

        def qk_unit(p, which, wtile, tt):
            def emit():
                key = (which, p)
                if key not in qkT:
                    qkT[key] = p_qk.tile([128, T], bf16, tag="qk", name=f"qk_{which}{p}")
                ps = ps_mm.tile([128, 512], f32, tag="mm")
                # fp8 DoubleRow: 2 contraction chunks per matmul (2 weights/cell)
                for cc in range(0, NCC, 2):
                    nc.tensor.matmul(
                        ps, wtile[:, cc : cc + 2, :],
                        xt8[:, cc : cc + 2, tt * 512 : (tt + 1) * 512],
                        start=(cc == 0), stop=(cc == NCC - 2), perf_mode=DR)
                nc.vector.tensor_copy(qkT[key][:, tt * 512 : (tt + 1) * 512], ps)
            return emit

        ctxT_store = {}  # (p, t) -> [128, 512] bf16 tile (normalized ctx^T)
        wo_tiles = {}

        def outproj_unit(t, qq):
            """out[t*512+qq*128 : +128, :] = sum_p ctxT[p,t][:,qq]^T @ wo_p."""
            def emit():
                stg = p_ostg.tile([128, 1024], bf16, tag="ostg")
                for half in range(2):
                    ps = ps_mm.tile([128, 512], f32, tag="mm")
                    for p in range(NPAIR):
                        nc.tensor.matmul(
                            ps, ctxT_store[(p, t)][:, qq * 128 : (qq + 1) * 128],
                            wo_tiles[p][:, half * 512 : (half + 1) * 512],
                            start=(p == 0), stop=(p == NPAIR - 1))
                    nc.vector.tensor_copy(stg[:, half * 512 : (half + 1) * 512], ps)
                nc.sync.dma_start(
                    out[t * 512 + qq * 128 : t * 512 + (qq + 1) * 128, :], stg)
            return emit

        pending_norm = []

        def make_norm(p, t, cxs):
            ct = p_ctxT.tile([128, 512], bf16, tag="ctxT", name=f"ct_{p}_{t}")
            ctxT_store[(p, t)] = ct
            def rep64(row):
                # [1,512] SBUF row -> [1, 64, 512] AP repeating the row 64x
                # (0-step on a free dim; partition dim must keep step!=0)
                return bass.AP(tensor=row.tensor, offset=row.offset,
                               ap=[list(row.ap[0]), [0, 64], list(row.ap[1])])
            state = {}
            def front():
                sc = p_small.tile([64, 16], bf16, tag="scat")
                rc = p_small.tile([128, 1024], bf16, tag="recip")
                bcab = p_small.tile([64, 1024], bf16, tag="bcast")
                bc = bcab[:, 0:512]
                bc2 = bcab[:, 512:1024]
                # scatter denom row over 64 lanes for the reciprocal
                # (serial per lane), gather, broadcast.
                nc.scalar.dma_start(sc, cxs[64:65, :])
                with nc.allow_low_precision(reason="bf16 softmax recip, tol 2e-2"):
                    nc.vector.reciprocal(sc, sc)
                nc.scalar.dma_start(rc[64:65, :], sc)
                nc.scalar.dma_start(bc, rep64(rc[64:65, 0:512]))
                nc.scalar.dma_start(bc2, rep64(rc[64:65, 512:1024]))
                state["bc"], state["bc2"] = bc, bc2
            def back():
                tmpB = p_small.tile([64, 512], bf16, tag="tmpB")
                nc.vector.tensor_mul(ct[0:64, :], cxs[0:64, 0:512], state["bc"])
                nc.vector.tensor_mul(tmpB, cxs[0:64, 512:1024], state["bc2"])
                nc.scalar.dma_start(ct[64:128, :], tmpB)
            return p, t, front, back

        # ---- attention for one pair, pulling filler units between exp groups ----
        def attention(p, qt, kt, filler):
            for t in range(NQT):
                nk = 4 * (t + 1)
                norms = list(pending_norm)
                pending_norm.clear()
                for _, _, fr, _ in norms:
                    fr()
                cx = ps_ctx.tile([128, 1024], f32, tag="ctx")
                ctxA = cx[:, 0:512]
                ctxB = cx[:, 512:1024]
                for j in range(nk):
                    if j == 2:
                        for pp, tt, _, bk in norms:
                            bk()
                            if pp == NPAIR - 1:
                                for qq in range(4):
                                    filler.append(outproj_unit(tt, qq))
                    # causal narrowing: columns q < off are fully masked for
                    # this k-tile -> skip them in S, exp and ctx.
                    off = max(0, j * 128 - t * 512)
                    W = 512 - off
                    qs = t * 512 + off
                    # S^T for both heads, row-tiled (contraction d=64 each)
                    sps = ps_s.tile([128, 1024], f32, tag="s")
                    nc.tensor.matmul(
                        sps[:, off : 512], kt[0:64, j * 128 : (j + 1) * 128],
                        qt[0:64, qs : (t + 1) * 512],
                        start=True, stop=True, tile_position=(0, 0))
                    nc.tensor.matmul(
                        sps[:, 512 + off : 1024], kt[64:128, j * 128 : (j + 1) * 128],
                        qt[64:128, qs : (t + 1) * 512],
                        start=True, stop=True, tile_position=(64, 0))
                    # exp(scale * S^T) for both heads in one ACT instruction
                    # ([128, 2, W] AP skips the masked prefix columns)
                    ph = p_phat.tile([128, 1024], bf16, tag="phat")
                    nc.scalar.activation(
                        ph.rearrange("p (h w) -> p h w", h=2)[:, :, off:512],
                        sps.rearrange("p (h w) -> p h w", h=2)[:, :, off:512],
                        EXP, scale=SCALE_S)
                    # causal zeroing on the 128-col diagonal slab (q in
                    # [off, off+128)): standard lower-triangular mask.
                    if j * 128 + 127 > t * 512:  # block crosses the diagonal
                        oe = min(off + 128, 512)
                        for h in range(2):
                            nc.gpsimd.affine_select(
                                out=ph[:, h * 512 + off : h * 512 + oe],
                                in_=ph[:, h * 512 + off : h * 512 + oe],
                                compare_op=mybir.AluOpType.is_ge,
                                fill=0.0, base=0,
                                pattern=[[1, oe - off]], channel_multiplier=-1)
                    # ctx'^T accumulation: one M=DV matmul per head gives
                    # rows 0:64 = ctx^T and row 64 = softmax denominator
                    # (V' ones column). Single accumulation group per bank.
                    st, sp = (j == 0), (j == nk - 1)
                    assert v_sb[j] is not None, f"V tile {j} not emitted yet"
                    nc.tensor.matmul(ctxA[0:DV, off:512], v_sb[j][:, 2 * p, :],
                                     ph[:, off : 512], start=st, stop=sp)
                    nc.tensor.matmul(ctxB[0:DV, off:512], v_sb[j][:, 2 * p + 1, :],
                                     ph[:, 512 + off : 1024], start=st, stop=sp)
                    if filler and j >= (3 if norms else 1):
                        filler.pop(0)()
                # Evict unnormalized ctx' (rows 0:64 ctx, row 64 denom) to
                # SBUF in ONE DVE copy so the psum bank frees immediately.
                # The multi-hop normalize is deferred into the NEXT q-tile
                # iteration (front half at its start, muls at its middle) so
                # its DMA latency never heads the DVE queue.
                cxs = p_cxs.tile([128, 1024], bf16, tag="cxs")
                nc.vector.tensor_copy(cxs[0:65, :], cx[0:65, :])
                pending_norm.append(make_norm(p, t, cxs))
                if filler:
                    filler.pop(0)()

        # ================= emission schedule =================
        # V tiles 0..3 + pair-0 Q/K tile 0 upfront; pair-0's later Q/K tiles,
        # the rest of V, later pairs' proj and the accumulated out-proj are
        # filler inside the attention loops (ordered so each is emitted
        # before its first use -- the v_sb/qkT asserts verify this).
        w0q, w0k = load_wqk(0)
        for j in range(4 * 1):
            v_unit(j)()
        qk_unit(0, "q", w0q, 0)()
        qk_unit(0, "k", w0k, 0)()

        for p in range(NPAIR):
            filler = []
            if p == 0:
                vq = [v_unit(j) for j in range(4, NKT)]
                for tt in range(1, NQT):
                    filler.append(qk_unit(0, "q", w0q, tt))
                    filler.append(qk_unit(0, "k", w0k, tt))
                    filler.extend(vq[:2])
                    vq = vq[2:]
                filler.extend(vq)
            if p + 1 < NPAIR:
                wq_t, wk_t = load_wqk(p + 1)
                for tt in range(NQT):
                    filler.append(qk_unit(p + 1, "q", wq_t, tt))
                    filler.append(qk_unit(p + 1, "k", wk_t, tt))
            wo_tiles[p] = load_wo(p)
            attention(p, qkT[("q", p)], qkT[("k", p)], filler)
            for u in filler:  # drain any leftovers
                u()
            qkT.pop(("q", p)), qkT.pop(("k", p))
        # tail: last tile's normalize + its out-projection
        for pp, tt, fr, bk in pending_norm:
            fr(); bk()
            if pp == NPAIR - 1:
                for qq in range(4):
                    outproj_unit(tt, qq)()
        pending_norm.clear()

    nc.compile()
    return nc


_NC_CACHE = {}


def _get_nc(T):
    if T not in _NC_CACHE:
        _NC_CACHE[T] = build_kernel(T)
    return _NC_CACHE[T]


def _bf16(a):
    import ml_dtypes
    return np.ascontiguousarray(a).astype(ml_dtypes.bfloat16)


def _fp8(a):
    import ml_dtypes
    return np.ascontiguousarray(a).astype(ml_dtypes.float8_e4m3)


def make_in_maps(x, W_q, W_k, W_v, W_o):
    B, T, _ = x.shape
    in_maps = []
    for c in range(8):
        b, g = c // 2, c % 2
        cols = slice(g * HG, (g + 1) * HG)
        xTb = np.asarray(x[b]).T
        in_maps.append({
            "xT": _bf16(xTb),
            "xT8": _fp8(xTb),
            "wq": _fp8(np.asarray(W_q)[:, cols] * WS),
            "wk": _fp8(np.asarray(W_k)[:, cols] * WS),
            "wv": _bf16(np.asarray(W_v)[:, cols]),
            "wo": _bf16(np.asarray(W_o)[cols, :]),
        })
    return in_maps


def kernel(x, W_q, W_k, W_v, W_o, b_o):
    x = np.asarray(x, dtype=np.float32)
    B, T, C_ = x.shape
    nc = _get_nc(T)
    in_maps = make_in_maps(x, W_q, W_k, W_v, W_o)
    res = run_bass_kernel_spmd(nc, in_maps, core_ids=list(range(8)))
    out = np.empty((B, T, C_), dtype=np.float32)
    bo = np.asarray(b_o, dtype=np.float32)[None, :]
    for b in range(B):
        pa = np.asarray(res.results[2 * b]["out"]).astype(np.float32)
        pb = np.asarray(res.results[2 * b + 1]["out"]).astype(np.float32)
        out[b] = pa + pb + bo
    return out


# revision 27
# speedup vs baseline: 1.5264x; 1.0327x over previous
"""Multi-head causal attention (B=4, T=2048, C=1024, H=16) on 8 TRN2 NeuronCores.

Sharding: core c handles batch b=c//2 and head-group g=c%2 (8 heads = 4 pairs).
Per core: QKV projections for its 512 feature columns, causal attention for its
8 heads, out-projection accumulated over the 4 head pairs in PSUM. Host sums
the two head-group partials per batch and adds b_o.

All operands are bf16 (1 cyc/row matmuls + FWL weight loads + half the DMA
bytes); accumulation stays fp32 in PSUM.
"""
import sys
import numpy as np
from contextlib import ExitStack

sys.path.insert(0, "/opt/trn_rl_repo")

import concourse.bass as bass
import concourse.tile as tile
from concourse import bacc, mybir
from concourse.bass_utils import run_bass_kernel_spmd

f32 = mybir.dt.float32
bf16 = mybir.dt.bfloat16
fp8 = mybir.dt.float8e4
DR = mybir.MatmulPerfMode.DoubleRow
EXP = mybir.ActivationFunctionType.Exp

C = 1024          # model dim
HG = 512          # per-core head-group feature width (8 heads x 64)
D = 64            # head dim
DV = 66           # V' row: 64 V cols + ones col + pad (even => 4B-aligned heads)
NPAIR = 4         # head pairs per core
NCC = C // 128    # contraction chunks (8)
WS = 32.0         # host-side W_q/W_k fp8 scale (absorbed into the exp scale)
SCALE = 0.125     # 1/sqrt(D)
SCALE_S = SCALE / (WS * WS)   # exp scale for fp8-scaled Q/K scores


def build_kernel(T):
    """Emit the per-core Bass program. T = sequence length (multiple of 512)."""
    NQT = T // 512    # q tiles of 512
    NKT = T // 128    # k tiles of 128

    nc = bacc.Bacc("TRN2", target_bir_lowering=False, debug=False, num_devices=8)

    xT = nc.dram_tensor("xT", [C, T], bf16, kind="ExternalInput").ap()
    wq = nc.dram_tensor("wq", [C, HG], fp8, kind="ExternalInput").ap()
    wk = nc.dram_tensor("wk", [C, HG], fp8, kind="ExternalInput").ap()
    wv = nc.dram_tensor("wv", [C, HG], bf16, kind="ExternalInput").ap()
    wo = nc.dram_tensor("wo", [HG, C], bf16, kind="ExternalInput").ap()
    out = nc.dram_tensor("out", [T, C], bf16, kind="ExternalOutput").ap()

    with tile.TileContext(nc) as tc, ExitStack() as ctx:
        # ---- SBUF pools (bytes/partition noted) ----
        p_xt = ctx.enter_context(tc.tile_pool(name="xt", bufs=NCC))          # 8x4K=32K
        p_w = ctx.enter_context(tc.tile_pool(name="w", bufs=2))              # qk pair weights 2x2x2K
        p_wv = ctx.enter_context(tc.tile_pool(name="wv", bufs=1))            # 8K
        p_wo = ctx.enter_context(tc.tile_pool(name="wo", bufs=NPAIR))        # 4x2K
        p_qk = ctx.enter_context(tc.tile_pool(name="qk", bufs=4))            # 4x4K
        p_v = ctx.enter_context(tc.tile_pool(name="v", bufs=4))              # 4x4.2K
        p_phat = ctx.enter_context(tc.tile_pool(name="phat", bufs=3))        # 3x2K
        p_ctxT = ctx.enter_context(tc.tile_pool(name="ctxT", bufs=4 * NPAIR))  # 16x1K
        p_cxs = ctx.enter_context(tc.tile_pool(name="cxs", bufs=3))          # 3x2K
        p_small = ctx.enter_context(tc.tile_pool(name="small", bufs=1))      # recip/bcast
        p_ostg = ctx.enter_context(tc.tile_pool(name="ostg", bufs=3))        # 3x2K
        p_ones = ctx.enter_context(tc.tile_pool(name="ones", bufs=1))
        # ---- PSUM pools: 4 + 2 + 2 = 8 banks ----
        ps_s = ctx.enter_context(tc.tile_pool(name="ps_s", bufs=2, space="PSUM"))    # [128,1024] x2
        ps_ctx = ctx.enter_context(tc.tile_pool(name="ps_ctx", bufs=1, space="PSUM"))
        ps_mm = ctx.enter_context(tc.tile_pool(name="ps_mm", bufs=2, space="PSUM"))

        # ---- constants + bulk loads ----
        ones_f = p_ones.tile([128, 1], bf16)
        nc.vector.memset(ones_f, 1.0)

        # HAM warm-up: ~5us of dummy matmuls during the input DMAs so the
        # PE clock is at 2.4GHz when real work starts.
        warm = p_ostg.tile([128, 512], bf16, tag="ostg")
        nc.vector.memset(warm, 0.0)
        wps = ps_mm.tile([128, 512], f32, tag="mm")
        for i in range(16):
            nc.tensor.matmul(wps, warm[:, 0:128], warm,
                             start=(i == 0), stop=(i == 15))

        wv_sb = p_wv.tile([128, NCC, HG], bf16)
        nc.sync.dma_start(wv_sb, wv.rearrange("(cc p) f -> p cc f", p=128))
        xt = []
        for cc in range(NCC):
            t_ = p_xt.tile([128, T], bf16, tag="xt")
            nc.sync.dma_start(t_, xT[cc * 128 : (cc + 1) * 128, :])
            xt.append(t_)
        # fp8 copy of x^T for the DoubleRow Q/K projections, cast on-chip
        # (saves 2MB of HBM input traffic during the startup ramp)
        xt8 = p_xt.tile([128, NCC, T], fp8, tag="xt8", bufs=1)
        for cc in range(NCC):
            nc.vector.tensor_copy(xt8[:, cc, :], xt[cc])

        def load_wqk(p):
            """[128, 8, 128] fp8 tile: cc-chunks of W{q,k}[:, p*128:(p+1)*128]."""
            tq = p_w.tile([128, NCC, 128], fp8, tag="wq")
            tk = p_w.tile([128, NCC, 128], fp8, tag="wk")
            nc.scalar.dma_start(
                tq, wq[:, p * 128 : (p + 1) * 128].rearrange("(cc p) f -> p cc f", p=128))
            nc.scalar.dma_start(
                tk, wk[:, p * 128 : (p + 1) * 128].rearrange("(cc p) f -> p cc f", p=128))
            return tq, tk

        def load_wo(p):
            t_ = p_wo.tile([128, C], bf16, tag="wo", name=f"wo{p}")
            nc.scalar.dma_start(t_, wo[p * 128 : (p + 1) * 128, :])
            return t_

        # ---- filler unit generators (PE work to hide under ACT-bound attention) ----
        v_groups = [None] * (NKT // 4)   # [128, 4, 8, DV] tiles, 4 k-tiles each

        def v_tile(j):
            g = v_groups[j // 4]
            assert g is not None, f"V group {j // 4} not emitted yet"
            return g[:, j % 4]

        v_sb = [None] * NKT

        def v_unit(j):
            st = {}
            def emit_a():
                st["ps"] = ps_mm.tile([128, HG], f32, tag="mm", name="vps")
                for cc in range(NCC // 2):
                    nc.tensor.matmul(
                        st["ps"], xt[cc][:, j * 128 : (j + 1) * 128],
                        wv_sb[:, cc, :], start=(cc == 0), stop=False)
            def emit_b():
                ps = st["ps"]
                for cc in range(NCC // 2, NCC):
                    nc.tensor.matmul(
                        ps, xt[cc][:, j * 128 : (j + 1) * 128],
                        wv_sb[:, cc, :], start=False, stop=(cc == NCC - 1))
                # V' layout [128, 8 heads, DV]: 64 V columns + a ones column so a
                # single M=65 ctx matmul also produces the softmax denominator.
                if j % 4 == 0:
                    v_groups[j // 4] = p_v.tile(
                        [128, 4, 8, DV], bf16, tag="v", name=f"vg{j // 4}")
                t_ = v_tile(j)
                nc.vector.tensor_copy(
                    t_[:, :, 0:D], ps.rearrange("p (h d) -> p h d", h=8))
                nc.vector.tensor_copy(
                    t_[:, :, D : DV], ones_f.to_broadcast([128, 8, DV - D]))
                v_sb[j] = t_
            return [emit_a, emit_b]

        qkT = {}   # (('q'|'k'), pair) -> [128, T] tile

        def qk_unit(p, which, wtile, tt):
            st = {}
            def emit_a():
                key = (which, p)
                if key not in qkT:
                    qkT[key] = p_qk.tile([128, T], bf16, tag="qk", name=f"qk_{which}{p}")
                st["ps"] = ps_mm.tile([128, 512], f32, tag="mm", name="qkps")
                # fp8 DoubleRow: 2 contraction chunks per matmul (2 weights/cell)
                for cc in (0, 2):
                    nc.tensor.matmul(
                        st["ps"], wtile[:, cc : cc + 2, :],
                        xt8[:, cc : cc + 2, tt * 512 : (tt + 1) * 512],
                        start=(cc == 0), stop=False, perf_mode=DR)
            def emit_b():
                ps = st["ps"]
                for cc in (4, 6):
                    nc.tensor.matmul(
                        ps, wtile[:, cc : cc + 2, :],
                        xt8[:, cc : cc + 2, tt * 512 : (tt + 1) * 512],
                        start=False, stop=(cc == 6), perf_mode=DR)
                nc.vector.tensor_copy(qkT[(which, p)][:, tt * 512 : (tt + 1) * 512], ps)
            return [emit_a, emit_b]

        ctxT_store = {}  # (p, t) -> [128, 512] bf16 tile (normalized ctx^T)
        wo_tiles = {}

        def outproj_unit(t, qq):
            """out[t*512+qq*128 : +128, :] = sum_p ctxT[p,t][:,qq]^T @ wo_p."""
            st = {}
            def half(h):
                ps = ps_mm.tile([128, 512], f32, tag="mm")
                for p in range(NPAIR):
                    nc.tensor.matmul(
                        ps, ctxT_store[(p, t)][:, qq * 128 : (qq + 1) * 128],
                        wo_tiles[p][:, h * 512 : (h + 1) * 512],
                        start=(p == 0), stop=(p == NPAIR - 1))
                nc.vector.tensor_copy(st["stg"][:, h * 512 : (h + 1) * 512], ps)
            def emit_a():
                st["stg"] = p_ostg.tile([128, 1024], bf16, tag="ostg", name="ostg")
                half(0)
            def emit_b():
                half(1)
                nc.sync.dma_start(
                    out[t * 512 + qq * 128 : t * 512 + (qq + 1) * 128, :], st["stg"])
            return [emit_a, emit_b]

        pending_norm = []

        def make_norm(p, t, cxs):
            ct = p_ctxT.tile([128, 512], bf16, tag="ctxT", name=f"ct_{p}_{t}")
            ctxT_store[(p, t)] = ct
            def rep64(row):
                # [1,512] SBUF row -> [1, 64, 512] AP repeating the row 64x
                # (0-step on a free dim; partition dim must keep step!=0)
                return bass.AP(tensor=row.tensor, offset=row.offset,
                               ap=[list(row.ap[0]), [0, 64], list(row.ap[1])])
            state = {}
            def front():
                sc = p_small.tile([64, 16], bf16, tag="scat")
                rc = p_small.tile([128, 1024], bf16, tag="recip")
                bcab = p_small.tile([64, 1024], bf16, tag="bcast")
                bc = bcab[:, 0:512]
                bc2 = bcab[:, 512:1024]
                # scatter denom row over 64 lanes for the reciprocal
                # (serial per lane), gather, broadcast.
                nc.scalar.dma_start(sc, cxs[64:65, :])
                with nc.allow_low_precision(reason="bf16 softmax recip, tol 2e-2"):
                    nc.vector.reciprocal(sc, sc)
                nc.scalar.dma_start(rc[64:65, :], sc)
                nc.scalar.dma_start(bc, rep64(rc[64:65, 0:512]))
                nc.scalar.dma_start(bc2, rep64(rc[64:65, 512:1024]))
                state["bc"], state["bc2"] = bc, bc2
            def back():
                tmpB = p_small.tile([64, 512], bf16, tag="tmpB")
                nc.vector.tensor_mul(ct[0:64, :], cxs[0:64, 0:512], state["bc"])
                nc.vector.tensor_mul(tmpB, cxs[0:64, 512:1024], state["bc2"])
                nc.scalar.dma_start(ct[64:128, :], tmpB)
            return p, t, front, back

        # ---- attention for one pair, pulling filler units between exp groups ----
        def attention(p, qt, kt, filler, t_order=None):
            for t in t_order or range(NQT):
                nk = 4 * (t + 1)
                norms = list(pending_norm)
                pending_norm.clear()
                for _, _, fr, _ in norms:
                    fr()
                cx = ps_ctx.tile([128, 1024], f32, tag="ctx")
                ctxA = cx[:, 0:512]
                ctxB = cx[:, 512:1024]
                for j in range(nk):
                    if j == 2:
                        for pp, tt, _, bk in norms:
                            bk()
                            if pp == NPAIR - 1:
                                for qq in range(4):
                                    filler.extend(outproj_unit(tt, qq))
                    # causal narrowing: columns q < off are fully masked for
                    # this k-tile -> skip them in S, exp and ctx.
                    off = max(0, j * 128 - t * 512)
                    W = 512 - off
                    qs = t * 512 + off
                    # S^T for both heads, row-tiled (contraction d=64 each)
                    sps = ps_s.tile([128, 1024], f32, tag="s")
                    nc.tensor.matmul(
                        sps[:, off : 512], kt[0:64, j * 128 : (j + 1) * 128],
                        qt[0:64, qs : (t + 1) * 512],
                        start=True, stop=True, tile_position=(0, 0))
                    nc.tensor.matmul(
                        sps[:, 512 + off : 1024], kt[64:128, j * 128 : (j + 1) * 128],
                        qt[64:128, qs : (t + 1) * 512],
                        start=True, stop=True, tile_position=(64, 0))
                    # exp(scale * S^T) for both heads in one ACT instruction
                    # ([128, 2, W] AP skips the masked prefix columns)
                    ph = p_phat.tile([128, 1024], bf16, tag="phat")
                    nc.scalar.activation(
                        ph.rearrange("p (h w) -> p h w", h=2)[:, :, off:512],
                        sps.rearrange("p (h w) -> p h w", h=2)[:, :, off:512],
                        EXP, scale=SCALE_S)
                    # causal zeroing on the 128-col diagonal slab (q in
                    # [off, off+128)): standard lower-triangular mask.
                    if j * 128 + 127 > t * 512:  # block crosses the diagonal
                        oe = min(off + 128, 512)
                        for h in range(2):
                            nc.gpsimd.affine_select(
                                out=ph[:, h * 512 + off : h * 512 + oe],
                                in_=ph[:, h * 512 + off : h * 512 + oe],
                                compare_op=mybir.AluOpType.is_ge,
                                fill=0.0, base=0,
                                pattern=[[1, oe - off]], channel_multiplier=-1)
                    # ctx'^T accumulation: one M=DV matmul per head gives
                    # rows 0:64 = ctx^T and row 64 = softmax denominator
                    # (V' ones column). Single accumulation group per bank.
                    st, sp = (j == 0), (j == nk - 1)
                    assert v_sb[j] is not None, f"V tile {j} not emitted yet"
                    nc.tensor.matmul(ctxA[0:DV, off:512], v_sb[j][:, 2 * p, :],
                                     ph[:, off : 512], start=st, stop=sp)
                    nc.tensor.matmul(ctxB[0:DV, off:512], v_sb[j][:, 2 * p + 1, :],
                                     ph[:, 512 + off : 1024], start=st, stop=sp)
                    if j >= (3 if norms else 1):
                        for _ in range(min(2, len(filler))):
                            filler.pop(0)()
                # Evict unnormalized ctx' (rows 0:64 ctx, row 64 denom) to
                # SBUF in ONE DVE copy so the psum bank frees immediately.
                # The multi-hop normalize is deferred into the NEXT q-tile
                # iteration (front half at its start, muls at its middle) so
                # its DMA latency never heads the DVE queue.
                cxs = p_cxs.tile([128, 1024], bf16, tag="cxs")
                nc.vector.tensor_copy(cxs[0:65, :], cx[0:65, :])
                pending_norm.append(make_norm(p, t, cxs))
                for _ in range(min(2, len(filler))):
                    filler.pop(0)()

        # ================= emission schedule =================
        # V tiles 0..3 + pair-0 Q/K tile 0 upfront; pair-0's later Q/K tiles,
        # the rest of V, later pairs' proj and the accumulated out-proj are
        # filler inside the attention loops (ordered so each is emitted
        # before its first use -- the v_sb/qkT asserts verify this).
        w0q, w0k = load_wqk(0)
        for j in range(4 * 1):
            for piece in v_unit(j):
                piece()
        for piece in qk_unit(0, "q", w0q, 0) + qk_unit(0, "k", w0k, 0):
            piece()

        for p in range(NPAIR):
            filler = []
            if p == 0:
                vq = [v_unit(j) for j in range(4, NKT)]
                for tt in range(1, NQT):
                    filler.extend(qk_unit(0, "q", w0q, tt))
                    filler.extend(qk_unit(0, "k", w0k, tt))
                    for u in vq[:2]:
                        filler.extend(u)
                    vq = vq[2:]
                for u in vq:
                    filler.extend(u)
            if p + 1 < NPAIR:
                wq_t, wk_t = load_wqk(p + 1)
                for tt in range(NQT):
                    filler.extend(qk_unit(p + 1, "q", wq_t, tt))
                    filler.extend(qk_unit(p + 1, "k", wk_t, tt))
            wo_tiles[p] = load_wo(p)
            # last pair ends on the smallest q-tile so the un-hideable
            # exp tail + final out-proj is as short as possible
            t_order = None
            if p == NPAIR - 1 and NQT >= 3:
                t_order = [NQT - 2, NQT - 1] + list(range(NQT - 3, -1, -1))
            attention(p, qkT[("q", p)], qkT[("k", p)], filler, t_order)
            for u in filler:  # drain any leftovers
                u()
            qkT.pop(("q", p)), qkT.pop(("k", p))
        # tail: last tile's normalize + its out-projection
        for pp, tt, fr, bk in pending_norm:
            fr(); bk()
            if pp == NPAIR - 1:
                for qq in range(4):
                    for piece in outproj_unit(tt, qq):
                        piece()
        pending_norm.clear()

    nc.compile()
    return nc


_NC_CACHE = {}


def _get_nc(T):
    if T not in _NC_CACHE:
        _NC_CACHE[T] = build_kernel(T)
    return _NC_CACHE[T]


def _bf16(a):
    import ml_dtypes
    return np.ascontiguousarray(a).astype(ml_dtypes.bfloat16)


def _fp8(a):
    import ml_dtypes
    return np.ascontiguousarray(a).astype(ml_dtypes.float8_e4m3)


def make_in_maps(x, W_q, W_k, W_v, W_o):
    B, T, _ = x.shape
    in_maps = []
    for c in range(8):
        b, g = c // 2, c % 2
        cols = slice(g * HG, (g + 1) * HG)
        xTb = np.asarray(x[b]).T
        in_maps.append({
            "xT": _bf16(xTb),
            "wq": _fp8(np.asarray(W_q)[:, cols] * WS),
            "wk": _fp8(np.asarray(W_k)[:, cols] * WS),
            "wv": _bf16(np.asarray(W_v)[:, cols]),
            "wo": _bf16(np.asarray(W_o)[cols, :]),
        })
    return in_maps


def kernel(x, W_q, W_k, W_v, W_o, b_o):
    x = np.asarray(x, dtype=np.float32)
    B, T, C_ = x.shape
    nc = _get_nc(T)
    in_maps = make_in_maps(x, W_q, W_k, W_v, W_o)
    res = run_bass_kernel_spmd(nc, in_maps, core_ids=list(range(8)))
    out = np.empty((B, T, C_), dtype=np.float32)
    bo = np.asarray(b_o, dtype=np.float32)[None, :]
    for b in range(B):
        pa = np.asarray(res.results[2 * b]["out"]).astype(np.float32)
        pb = np.asarray(res.results[2 * b + 1]["out"]).astype(np.float32)
        out[b] = pa + pb + bo
    return out
